# revision 9
# baseline (speedup 1.0000x reference)
"""Trainium2 Bass kernel for nn_ChebConvNet (ChebConv K=1 => 3-layer MLP + log_softmax).

Computation per node row (edge_index is inert for K=1 ChebConv):
    h = silu(x @ W0 + b0); h = silu(h @ W1 + b1); h2 = h @ W2 + b2
    out = log_softmax(h2, axis=1)

Strategy: shard the 500k node rows across 8 NeuronCores (row-parallel, no
communication). On each core:
  - SWDGE cast-DMA loads x tiles fp32->bf16 row-major.
  - PE transposes 128x128 bf16 chunks to get feature-major x.T (matmul
    contraction must be over features, which requires feature-on-partitions).
  - mm1/mm2 stream activations (weights stationary, bf16, fp32 PSUM accum),
    SiLU on ACT reads PSUM and writes bf16 SBUF (bias folded into ACT bias).
  - mm3 uses h1 chunks as the stationary operand so h2 comes out row-major in
    PSUM; log_softmax then reduces along the free dim.
  - h2 parks in SBUF so exp/ln ACT ops batch into few table-set phases
    (SiLU and Exp/Ln live in different ACT table sets; switches cost ~2.7us).
  - exp without max-subtraction (h2 is O(+-8) here; fp32 exp is exact enough),
    DVE reduce + broadcast subtract, one big row-major store per 2048 rows.
"""

import math
import numpy as np
import ml_dtypes

import bass_rust
import concourse.bass as bass
import concourse.tile as tile
from concourse import mybir
from concourse.bass_utils import run_bass_kernel_spmd
from concourse.vector_clock import ScopedClock
from bass_rust import add_dep_helper

N_CORES = 8
F_IN = 128
F_HID = 128
F_OUT = 64
IT_ROWS = 512          # rows per pipeline iteration
MACRO_IT = 2           # iterations per PSUM macro (1024 rows)
LOAD_MACROS = 2        # macros per DMA load batch (2048 rows)
BATCH_ROWS = IT_ROWS * MACRO_IT * LOAD_MACROS  # 2048
PHASEB_CHUNK = 2048    # free elems per exp/sub op (= 4096 rows)

_DT = mybir.dt

# this walrus build rejects instructions with more than ONE sync wait; the
# Tile framework freely assigns several. Two patches below: (1) split every
# multi-wait instruction by inserting single-wait NoOp carriers on the same
# engine right before it (order on the engine's sequencer preserves
# semantics); (2) the TileContext tail drain gets the same treatment with
# single-wait drain carriers.
_MAX_DRAIN_WAITS = 1
_N_SPARE_DRAINS = 31

_NOOP_CLS = None
_carrier_counter = [0]


def _noop_cls():
    global _NOOP_CLS
    if _NOOP_CLS is None:
        _NOOP_CLS = getattr(bass_rust, "InstNoOp")
    return _NOOP_CLS


_orig_lower_ordered = tile.TileContext._lower_ordered_insts


def _split_multi_waits(self, ordered):
    cls = _noop_cls()
    new_ordered = {}
    for bb_name, insts in ordered.items():
        new_list = []
        for inst in insts:
            si = inst.sync_info
            waits = list(si.on_wait) if si is not None else []
            if len(waits) > 1:
                for w in waits[:-1]:
                    c = cls(name=f"waitcar-{_carrier_counter[0]}", ins=[],
                            outs=[])
                    _carrier_counter[0] += 1
                    c.engine = inst.engine
                    c.sync_info = bass_rust.SyncInfo(on_wait=[w], on_update=[])
                    new_list.append(c)
                inst.sync_info = bass_rust.SyncInfo(
                    on_wait=[waits[-1]], on_update=list(si.on_update))
            new_list.append(inst)
        new_ordered[bb_name] = new_list
    return _orig_lower_ordered(self, new_ordered)


tile.TileContext._lower_ordered_insts = _split_multi_waits


def _patched_drain_and_barrier(self, tick_clock, wait_clock):
    nc = self.nc
    spare = [nc.sync.drain() for _ in range(_N_SPARE_DRAINS)]
    drain_inst = nc.sync.drain()
    wait_clock.add_sem_waits(
        drain_inst.ins, ScopedClock({None: tick_clock.global_clock})
    )
    si = drain_inst.ins.sync_info
    waits = list(si.on_wait) if si is not None else []
    if len(waits) > _MAX_DRAIN_WAITS:
        chunks = [
            waits[i : i + _MAX_DRAIN_WAITS]
            for i in range(0, len(waits), _MAX_DRAIN_WAITS)
        ]
        head, tail = chunks[:-1], chunks[-1]
        assert len(head) <= _N_SPARE_DRAINS, "bump _N_SPARE_DRAINS"
        for nop_i, chunk in zip(spare, head):
            nop_i.ins.sync_info = bass_rust.SyncInfo(on_wait=chunk, on_update=[])
        drain_inst.ins.sync_info = bass_rust.SyncInfo(
            on_wait=tail, on_update=list(si.on_update)
        )
    nc.all_engine_barrier()
    assert self.sems is not None
    popped = nc._tile_sem_poison_stack.pop()
    assert popped is self._sem_poison
    nc.clear_and_free_semaphores(list(self.sems.allocated().values()))
    nc.all_engine_barrier()


tile.TileContext._drain_and_barrier = _patched_drain_and_barrier


def _build(nc_rows: int, with_b2: bool, sb_macros: int):
    """Build the per-core Bass module. nc_rows must be a multiple of 2048."""
    assert nc_rows % BATCH_ROWS == 0
    nc = bass.Bass("TRN2", target_bir_lowering=False, debug=False,
                   num_devices=N_CORES)

    x_d = nc.dram_tensor("x", [nc_rows, F_IN], _DT.float32,
                         kind="ExternalInput").ap()
    w0_d = nc.dram_tensor("w0", [F_IN, F_HID], _DT.bfloat16,
                          kind="ExternalInput").ap()
    w1_d = nc.dram_tensor("w1", [F_HID, F_HID], _DT.bfloat16,
                          kind="ExternalInput").ap()
    w2_d = nc.dram_tensor("w2", [F_HID, F_OUT], _DT.bfloat16,
                          kind="ExternalInput").ap()
    b0_d = nc.dram_tensor("b0", [F_HID, 1], _DT.float32,
                          kind="ExternalInput").ap()
    b1_d = nc.dram_tensor("b1", [F_HID, 1], _DT.float32,
                          kind="ExternalInput").ap()
    b2_d = nc.dram_tensor("b2", [1, F_OUT], _DT.bfloat16,
                          kind="ExternalInput").ap()
    id_d = nc.dram_tensor("ident", [128, 128], _DT.bfloat16,
                          kind="ExternalInput").ap()
    out_d = nc.dram_tensor("out", [nc_rows, F_OUT], _DT.float32,
                           kind="ExternalOutput").ap()

    n_macros = nc_rows // (IT_ROWS * MACRO_IT)
    AF = mybir.ActivationFunctionType

    with tile.TileContext(nc) as tc:
        with (
            tc.tile_pool(name="consts", bufs=1) as consts,
            tc.tile_pool(name="xb", bufs=4) as xpool,
            tc.tile_pool(name="xt_ps", bufs=1, space="PSUM") as xtp,
            tc.tile_pool(name="xt_sb", bufs=2) as xts,
            tc.tile_pool(name="h0_ps", bufs=2, space="PSUM") as h0p,
            tc.tile_pool(name="h1_ps", bufs=1, space="PSUM") as h1p,
            tc.tile_pool(name="h0_sb", bufs=3) as h0s,
            tc.tile_pool(name="h1_sb", bufs=3) as h1s,
            tc.tile_pool(name="h2_ps", bufs=1, space="PSUM") as h2p,
            tc.tile_pool(name="park", bufs=2) as parkp,
            tc.tile_pool(name="e", bufs=3) as epool,
            tc.tile_pool(name="s", bufs=2) as spool,
            tc.tile_pool(name="o", bufs=3) as opool,
        ):
            w0 = consts.tile([128, F_HID], _DT.bfloat16, tag="w0")
            nc.sync.dma_start(w0[:], w0_d[:, :])
            w1 = consts.tile([128, F_HID], _DT.bfloat16, tag="w1")
            nc.sync.dma_start(w1[:], w1_d[:, :])
            w2 = consts.tile([128, F_OUT], _DT.bfloat16, tag="w2")
            nc.sync.dma_start(w2[:], w2_d[:, :])
            b0 = consts.tile([128, 1], _DT.float32, tag="b0")
            nc.sync.dma_start(b0[:], b0_d[:, :])
            b1 = consts.tile([128, 1], _DT.float32, tag="b1")
            nc.sync.dma_start(b1[:], b1_d[:, :])
            ident = consts.tile([128, 128], _DT.bfloat16, tag="ident")
            nc.sync.dma_start(ident[:], id_d[:, :])
            b2 = None
            ones1 = None
            if with_b2:
                b2 = consts.tile([1, F_OUT], _DT.bfloat16, tag="b2")
                nc.sync.dma_start(b2[:], b2_d[:, :])
                ones1 = consts.tile([1, 128], _DT.bfloat16, tag="ones1")
                nc.gpsimd.memset(ones1[:], 1.0)

            # chain all ACT instructions in emission order so the scheduler
            # cannot interleave exp/ln (natural_log set) between silu ops
            last_act = [None]

            def act_order(bi):
                if last_act[0] is not None:
                    add_dep_helper(bi.ins, last_act[0].ins, sync=False,
                                   reason="act-table-set phase order")
                last_act[0] = bi

            def phase_b(pk, base_row, width):
                """softmax tail for parked h2: width free elems (64/row-blk).
                All exps run back-to-back on ACT (no DVE waits in the chain),
                then one batched ln, then GPSIMD subtracts + paired stores."""
                nblk_tot = width // F_OUT
                chunks = []
                off = 0
                while off < width:
                    w = min(PHASEB_CHUNK, width - off)
                    chunks.append((off, w))
                    off += w
                S = spool.tile([128, sb_macros * 8], _DT.float32, tag="s")
                LZ = spool.tile([128, sb_macros * 8], _DT.float32, tag="lz")
                for off, w in chunks:
                    nblk = w // F_OUT
                    e = epool.tile([128, PHASEB_CHUNK], _DT.float32, tag="e")
                    act_order(nc.scalar.activation(
                        e[:, :w], pk[:, off:off + w], AF.Exp))
                    nc.vector.tensor_reduce(
                        S[:, off // F_OUT:off // F_OUT + nblk],
                        e[:, :w].rearrange("p (b f) -> p b f", f=F_OUT),
                        axis=mybir.AxisListType.X, op=mybir.AluOpType.add)
                act_order(nc.scalar.activation(
                    LZ[:, :nblk_tot], S[:, :nblk_tot], AF.Ln))
                for off, w in chunks:
                    nblk = w // F_OUT
                    o = opool.tile([128, PHASEB_CHUNK], _DT.float32, tag="o")
                    lzb = (LZ[:, off // F_OUT:off // F_OUT + nblk]
                           .broadcast_to([128, nblk, F_OUT]))
                    sub_engine = nc.gpsimd if hasattr(nc.gpsimd, "tensor_tensor") \
                        else nc.vector
                    sub_engine.tensor_tensor(
                        out=o[:, :w].rearrange("p (b f) -> p b f", f=F_OUT),
                        in0=pk[:, off:off + w].rearrange(
                            "p (b f) -> p b f", f=F_OUT),
                        in1=lzb, op=mybir.AluOpType.subtract)
                    # paired-row layout: block B = 2*P + s holds rows
                    # row0 + 256*P + 2*q + s; (s, f) is 512B-contiguous in DRAM
                    row0 = base_row + (off // F_OUT) * 128
                    nrows = nblk * 128
                    nc.sync.dma_start(
                        out_d[row0:row0 + nrows, :].rearrange(
                            "(P q s) f -> q P s f", q=128, s=2),
                        o[:, :w].rearrange("p (P s f) -> p P s f",
                                           s=2, f=F_OUT))

            xb = None
            for m in range(n_macros):
                sb_idx = m % sb_macros
                if sb_idx == 0:
                    n_sb = min(sb_macros, n_macros - m)
                    pk = parkp.tile([128, sb_macros * 512], _DT.float32,
                                    tag="park")
                    sb_base_row = m * IT_ROWS * MACRO_IT

                if m % LOAD_MACROS == 0:
                    xb = xpool.tile([128, BATCH_ROWS], _DT.bfloat16, tag="xb")
                    r0 = m * IT_ROWS * MACRO_IT
                    nc.gpsimd.dma_start(
                        xb[:].rearrange("p (g f) -> p g f", f=F_IN),
                        x_d[r0:r0 + BATCH_ROWS, :].rearrange(
                            "(g p) f -> p g f", p=128))
                ml = m % LOAD_MACROS  # macro index within load batch

                h0t = h0p.tile([128, 1024], _DT.float32, tag="h0t")
                h1t = h1p.tile([128, 1024], _DT.float32, tag="h1t")
                h0b = h0s.tile([128, 1024], _DT.bfloat16, tag="h0b")
                h1b = h1s.tile([128, 1024], _DT.bfloat16, tag="h1b")
                h2t = h2p.tile([128, 512], _DT.float32, tag="h2t")

                for j in range(MACRO_IT):
                    xt_ps = xtp.tile([128, 512], _DT.bfloat16, tag="xt_ps")
                    for c in range(4):
                        g = ml * 8 + j * 4 + c
                        nc.tensor.transpose(
                            xt_ps[:, c * 128:(c + 1) * 128],
                            xb[:, g * 128:(g + 1) * 128],
                            ident[:])
                    xt = xts.tile([128, 512], _DT.bfloat16, tag="xt")
                    nc.vector.tensor_copy(xt[:], xt_ps[:])
                    nc.tensor.matmul(
                        h0t[:, j * 512:(j + 1) * 512], lhsT=w0[:], rhs=xt[:],
                        start=True, stop=True)

                act_order(nc.scalar.activation(
                    h0b[:], h0t[:], AF.Silu, bias=b0[:, 0:1]))

                for j in range(MACRO_IT):
                    nc.tensor.matmul(
                        h1t[:, j * 512:(j + 1) * 512], lhsT=w1[:],
                        rhs=h0b[:, j * 512:(j + 1) * 512],
                        start=True, stop=True)

                act_order(nc.scalar.activation(
                    h1b[:], h1t[:], AF.Silu, bias=b1[:, 0:1]))

                # mm3 with row-PAIRING: block b = (j, c2, s) covers rows
                # {512j + 256*c2 + 2q + s : q in 0..127}; adjacent s-blocks
                # make each partition's two rows CONSECUTIVE in DRAM, so the
                # store uses 512-byte descriptors instead of 256-byte ones.
                n_mm3 = MACRO_IT * 4 * (2 if with_b2 else 1)
                k = 0
                for j in range(MACRO_IT):
                    for c2 in range(2):
                        for s in range(2):
                            b = j * 4 + c2 * 2 + s
                            lview = (h1b[:, j * 512 + c2 * 256:
                                          j * 512 + (c2 + 1) * 256]
                                     .rearrange("p (q two) -> p q two", two=2)
                                     [:, :, s])
                            nc.tensor.matmul(
                                h2t[:, b * 64:(b + 1) * 64],
                                lhsT=lview, rhs=w2[:],
                                start=(k == 0), stop=(k == n_mm3 - 1))
                            k += 1
                if with_b2:
                    for b in range(MACRO_IT * 4):
                        nc.tensor.matmul(
                            h2t[:, b * 64:(b + 1) * 64],
                            lhsT=ones1[:], rhs=b2[:],
                            start=False, stop=(k == n_mm3 - 1))
                        k += 1

                nc.vector.tensor_copy(pk[:, sb_idx * 512:(sb_idx + 1) * 512],
                                      h2t[:])

                if sb_idx == n_sb - 1:
                    phase_b(pk, sb_base_row, n_sb * 512)

    return nc


_BUILD_CACHE = {}


def _get_module(nc_rows: int, with_b2: bool, sb_macros: int):
    key = (nc_rows, with_b2, sb_macros)
    if key not in _BUILD_CACHE:
        _BUILD_CACHE[key] = _build(nc_rows, with_b2, sb_macros)
    return _BUILD_CACHE[key]


def kernel(x, edge_index=None, W0=None, b0=None, W1=None, b1=None, W2=None,
           b2=None, **_unused):
    x = np.ascontiguousarray(np.asarray(x), dtype=np.float32)
    n = x.shape[0]
    per = int(math.ceil(n / N_CORES / BATCH_ROWS)) * BATCH_ROWS
    total = per * N_CORES

    xp = np.zeros((total, F_IN), dtype=np.float32)
    xp[:n] = x

    bf = ml_dtypes.bfloat16
    w0b = np.ascontiguousarray(np.asarray(W0, dtype=np.float32)).astype(bf)
    w1b = np.ascontiguousarray(np.asarray(W1, dtype=np.float32)).astype(bf)
    w2b = np.ascontiguousarray(np.asarray(W2, dtype=np.float32)).astype(bf)
    b0f = np.asarray(b0, dtype=np.float32).reshape(F_HID, 1)
    b1f = np.asarray(b1, dtype=np.float32).reshape(F_HID, 1)
    b2f = np.asarray(b2, dtype=np.float32).reshape(1, F_OUT)
    with_b2 = bool(np.any(b2f))
    b2b = b2f.astype(bf)
    ident = np.eye(128, dtype=bf)

    n_macros = per // (IT_ROWS * MACRO_IT)
    sb_macros = min(21, n_macros)

    nc = _get_module(per, with_b2, sb_macros)

    in_maps = []
    for i in range(N_CORES):
        in_maps.append({
            "x": xp[i * per:(i + 1) * per],
            "w0": w0b, "w1": w1b, "w2": w2b,
            "b0": b0f, "b1": b1f, "b2": b2b,
            "ident": ident,
        })

    res = run_bass_kernel_spmd(nc, in_maps, list(range(N_CORES)))
    out = np.concatenate([res.results[i]["out"] for i in range(N_CORES)],
                         axis=0)
    return np.ascontiguousarray(out[:n])


# revision 24
# speedup vs baseline: 415.9460x; 415.9460x over previous
"""Trainium2 Bass kernel for nn_ChebConvNet (ChebConv K=1 => 3-layer MLP + log_softmax).

Computation per node row (edge_index is inert for K=1 ChebConv):
    h = silu(x @ W0 + b0); h = silu(h @ W1 + b1); h2 = h @ W2 + b2
    out = log_softmax(h2, axis=1)

Strategy: shard the 500k node rows across 8 NeuronCores (row-parallel, no
communication). On each core:
  - SWDGE cast-DMA loads x tiles fp32->bf16 row-major.
  - PE transposes 128x128 bf16 chunks to get feature-major x.T (matmul
    contraction must be over features, which requires feature-on-partitions).
  - mm1/mm2 stream activations (weights stationary, bf16, fp32 PSUM accum),
    SiLU on ACT reads PSUM and writes bf16 SBUF (bias folded into ACT bias).
  - mm3 uses h1 chunks as the stationary operand so h2 comes out row-major in
    PSUM; log_softmax then reduces along the free dim.
  - h2 parks in SBUF so exp/ln ACT ops batch into few table-set phases
    (SiLU and Exp/Ln live in different ACT table sets; switches cost ~2.7us).
  - exp without max-subtraction (h2 is O(+-8) here; fp32 exp is exact enough),
    DVE reduce + broadcast subtract, one big row-major store per 2048 rows.
"""

import math
import numpy as np
import ml_dtypes

import bass_rust
import concourse.bass as bass
import concourse.tile as tile
from concourse import mybir
from concourse.bass_utils import run_bass_kernel_spmd
from concourse.vector_clock import ScopedClock
from bass_rust import add_dep_helper

N_CORES = 8
F_IN = 128
F_HID = 128
F_OUT = 64
IT_ROWS = 512          # rows per pipeline iteration
MACRO_IT = 2           # iterations per PSUM macro (1024 rows)
LOAD_MACROS = 2        # macros per DMA load batch (2048 rows)
BATCH_ROWS = IT_ROWS * MACRO_IT * LOAD_MACROS  # 2048
MACRO_ROWS = IT_ROWS * MACRO_IT
CHUNKS_PER_MACRO = MACRO_ROWS // 128       # 128-row transpose chunks
MACRO_FREE = CHUNKS_PER_MACRO * F_OUT      # h2/park free elems per macro
MACRO_BLKS = CHUNKS_PER_MACRO              # 64-wide row blocks per macro
PHASEB_CHUNK = 2048    # free elems per exp/sub op (= 4096 rows)

_DT = mybir.dt

# this walrus build rejects instructions with more than ONE sync wait; the
# Tile framework freely assigns several. Two patches below: (1) split every
# multi-wait instruction by inserting single-wait NoOp carriers on the same
# engine right before it (order on the engine's sequencer preserves
# semantics); (2) the TileContext tail drain gets the same treatment with
# single-wait drain carriers.
_MAX_DRAIN_WAITS = 1
_N_SPARE_DRAINS = 31

_NOOP_CLS = None
_carrier_counter = [0]


def _noop_cls():
    global _NOOP_CLS
    if _NOOP_CLS is None:
        _NOOP_CLS = getattr(bass_rust, "InstNoOp")
    return _NOOP_CLS


_orig_lower_ordered = tile.TileContext._lower_ordered_insts


def _split_multi_waits(self, ordered):
    cls = _noop_cls()
    new_ordered = {}
    for bb_name, insts in ordered.items():
        new_list = []
        for inst in insts:
            si = inst.sync_info
            waits = list(si.on_wait) if si is not None else []
            if len(waits) > 1:
                for w in waits[:-1]:
                    c = cls(name=f"waitcar-{_carrier_counter[0]}", ins=[],
                            outs=[])
                    _carrier_counter[0] += 1
                    c.engine = inst.engine
                    c.sync_info = bass_rust.SyncInfo(on_wait=[w], on_update=[])
                    new_list.append(c)
                inst.sync_info = bass_rust.SyncInfo(
                    on_wait=[waits[-1]], on_update=list(si.on_update))
            new_list.append(inst)
        new_ordered[bb_name] = new_list
    return _orig_lower_ordered(self, new_ordered)


tile.TileContext._lower_ordered_insts = _split_multi_waits


def _patched_drain_and_barrier(self, tick_clock, wait_clock):
    nc = self.nc
    spare = [nc.sync.drain() for _ in range(_N_SPARE_DRAINS)]
    drain_inst = nc.sync.drain()
    wait_clock.add_sem_waits(
        drain_inst.ins, ScopedClock({None: tick_clock.global_clock})
    )
    si = drain_inst.ins.sync_info
    waits = list(si.on_wait) if si is not None else []
    if len(waits) > _MAX_DRAIN_WAITS:
        chunks = [
            waits[i : i + _MAX_DRAIN_WAITS]
            for i in range(0, len(waits), _MAX_DRAIN_WAITS)
        ]
        head, tail = chunks[:-1], chunks[-1]
        assert len(head) <= _N_SPARE_DRAINS, "bump _N_SPARE_DRAINS"
        for nop_i, chunk in zip(spare, head):
            nop_i.ins.sync_info = bass_rust.SyncInfo(on_wait=chunk, on_update=[])
        drain_inst.ins.sync_info = bass_rust.SyncInfo(
            on_wait=tail, on_update=list(si.on_update)
        )
    nc.all_engine_barrier()
    assert self.sems is not None
    popped = nc._tile_sem_poison_stack.pop()
    assert popped is self._sem_poison
    nc.clear_and_free_semaphores(list(self.sems.allocated().values()))
    nc.all_engine_barrier()


tile.TileContext._drain_and_barrier = _patched_drain_and_barrier


def _build(nc_rows: int, with_b2: bool, sb_macros: int):
    """Build the per-core Bass module. nc_rows must be a multiple of 2048."""
    assert nc_rows % BATCH_ROWS == 0
    nc = bass.Bass("TRN2", target_bir_lowering=False, debug=False,
                   num_devices=N_CORES)

    x_d = nc.dram_tensor("x", [nc_rows, F_IN], _DT.float32,
                         kind="ExternalInput").ap()
    w0_d = nc.dram_tensor("w0", [F_IN, F_HID], _DT.bfloat16,
                          kind="ExternalInput").ap()
    w1_d = nc.dram_tensor("w1", [F_HID, F_HID], _DT.bfloat16,
                          kind="ExternalInput").ap()
    w2_d = nc.dram_tensor("w2", [F_HID, F_OUT], _DT.bfloat16,
                          kind="ExternalInput").ap()
    b0_d = nc.dram_tensor("b0", [F_HID, 1], _DT.float32,
                          kind="ExternalInput").ap()
    b1_d = nc.dram_tensor("b1", [F_HID, 1], _DT.float32,
                          kind="ExternalInput").ap()
    b2_d = nc.dram_tensor("b2", [1, F_OUT], _DT.bfloat16,
                          kind="ExternalInput").ap()
    id_d = nc.dram_tensor("ident", [128, 128], _DT.bfloat16,
                          kind="ExternalInput").ap()
    out_d = nc.dram_tensor("out", [nc_rows, F_OUT], _DT.float32,
                           kind="ExternalOutput").ap()

    n_macros = nc_rows // (IT_ROWS * MACRO_IT)
    AF = mybir.ActivationFunctionType

    with tile.TileContext(nc) as tc:
        with (
            tc.tile_pool(name="consts", bufs=1) as consts,
            tc.tile_pool(name="xb", bufs=4) as xpool,
            tc.tile_pool(name="xt_ps", bufs=2, space="PSUM") as xtp,
            tc.tile_pool(name="xt_sb", bufs=2) as xts,
            tc.tile_pool(name="h0_ps", bufs=1, space="PSUM") as h0p,
            tc.tile_pool(name="h1_ps", bufs=1, space="PSUM") as h1p,
            tc.tile_pool(name="h0_sb", bufs=3) as h0s,
            tc.tile_pool(name="h1_sb", bufs=3) as h1s,
            tc.tile_pool(name="h2_ps", bufs=2, space="PSUM") as h2p,
            tc.tile_pool(name="park", bufs=2) as parkp,
            tc.tile_pool(name="e", bufs=2) as epool,
            tc.tile_pool(name="s", bufs=2) as spool,
            tc.tile_pool(name="o", bufs=2) as opool,
        ):
            # ident first: the first PE transposes need it, and HWDGE
            # descriptor generation is serial (~0.6us per dma_start)
            ident = consts.tile([128, 128], _DT.bfloat16, tag="ident")
            nc.sync.dma_start(ident[:], id_d[:, :])
            w0 = consts.tile([128, F_HID], _DT.bfloat16, tag="w0")
            nc.sync.dma_start(w0[:], w0_d[:, :])
            b0 = consts.tile([128, 1], _DT.float32, tag="b0")
            nc.sync.dma_start(b0[:], b0_d[:, :])
            w1 = consts.tile([128, F_HID], _DT.bfloat16, tag="w1")
            nc.sync.dma_start(w1[:], w1_d[:, :])
            b1 = consts.tile([128, 1], _DT.float32, tag="b1")
            nc.sync.dma_start(b1[:], b1_d[:, :])
            w2 = consts.tile([128, F_OUT], _DT.bfloat16, tag="w2")
            nc.sync.dma_start(w2[:], w2_d[:, :])
            b2 = None
            ones1 = None
            if with_b2:
                b2 = consts.tile([1, F_OUT], _DT.bfloat16, tag="b2")
                nc.sync.dma_start(b2[:], b2_d[:, :])
                ones1 = consts.tile([1, 128], _DT.bfloat16, tag="ones1")
                nc.gpsimd.memset(ones1[:], 1.0)

            # chain all ACT instructions in emission order so the scheduler
            # cannot interleave exp/ln (natural_log set) between silu ops
            last_act = [None]

            def act_order(bi):
                if last_act[0] is not None:
                    add_dep_helper(bi.ins, last_act[0].ins, sync=False,
                                   reason="act-table-set phase order")
                last_act[0] = bi

            def phase_b(pk, base_row, width):
                """softmax tail for parked h2: width free elems (64/row-blk).
                All exps run back-to-back on ACT (no DVE waits in the chain),
                then one batched ln, then GPSIMD subtracts + paired stores."""
                nblk_tot = width // F_OUT
                chunks = []
                off = 0
                while off < width:
                    w = min(PHASEB_CHUNK, width - off)
                    chunks.append((off, w))
                    off += w
                S = spool.tile([128, sb_macros * MACRO_BLKS], _DT.float32, tag="s")
                LZ = spool.tile([128, sb_macros * MACRO_BLKS], _DT.float32, tag="lz")
                for off, w in chunks:
                    nblk = w // F_OUT
                    e = epool.tile([128, PHASEB_CHUNK], _DT.float32, tag="e")
                    act_order(nc.scalar.activation(
                        e[:, :w], pk[:, off:off + w], AF.Exp))
                    nc.vector.tensor_reduce(
                        S[:, off // F_OUT:off // F_OUT + nblk],
                        e[:, :w].rearrange("p (b f) -> p b f", f=F_OUT),
                        axis=mybir.AxisListType.X, op=mybir.AluOpType.add)
                act_order(nc.scalar.activation(
                    LZ[:, :nblk_tot], S[:, :nblk_tot], AF.Ln))
                for off, w in chunks:
                    nblk = w // F_OUT
                    o = opool.tile([128, PHASEB_CHUNK], _DT.float32, tag="o")
                    lzb = (LZ[:, off // F_OUT:off // F_OUT + nblk]
                           .broadcast_to([128, nblk, F_OUT]))
                    sub_engine = nc.gpsimd if hasattr(nc.gpsimd, "tensor_tensor") \
                        else nc.vector
                    sub_engine.tensor_tensor(
                        out=o[:, :w].rearrange("p (b f) -> p b f", f=F_OUT),
                        in0=pk[:, off:off + w].rearrange(
                            "p (b f) -> p b f", f=F_OUT),
                        in1=lzb, op=mybir.AluOpType.subtract)
                    # paired-row layout: block B = 2*P + s holds rows
                    # row0 + 256*P + 2*q + s; (s, f) is 512B-contiguous in DRAM
                    row0 = base_row + (off // F_OUT) * 128
                    nrows = nblk * 128
                    nc.sync.dma_start(
                        out_d[row0:row0 + nrows, :].rearrange(
                            "(P q s) f -> q P s f", q=128, s=2),
                        o[:, :w].rearrange("p (P s f) -> p P s f",
                                           s=2, f=F_OUT))

            # superbatch schedule: full-size SBs, then a short final SB so
            # the last (un-overlapped) phase-B tail is small
            TAIL_SB = max(2, sb_macros // 3)
            sb_sizes = []
            rem = n_macros
            while rem > 0:
                if rem <= sb_macros:
                    sb_sizes.append(rem); rem = 0
                elif rem <= sb_macros + TAIL_SB:
                    sb_sizes.append(rem - TAIL_SB); sb_sizes.append(TAIL_SB)
                    rem = 0
                else:
                    sb_sizes.append(sb_macros); rem -= sb_macros
            sb_bounds = []
            acc = 0
            for sz in sb_sizes:
                sb_bounds.append((acc, sz)); acc += sz
            sb_start = {st: sz for st, sz in sb_bounds}

            xb = None
            sb_idx = 0
            for m in range(n_macros):
                if m in sb_start:
                    n_sb = sb_start[m]
                    sb_idx = 0
                    pk = parkp.tile([128, sb_macros * MACRO_FREE], _DT.float32,
                                    tag="park")
                    sb_base_row = m * IT_ROWS * MACRO_IT

                if m % LOAD_MACROS == 0:
                    xb = xpool.tile([128, BATCH_ROWS], _DT.bfloat16, tag="xb")
                    r0 = m * IT_ROWS * MACRO_IT
                    nc.gpsimd.dma_start(
                        xb[:].rearrange("p (g f) -> p g f", f=F_IN),
                        x_d[r0:r0 + BATCH_ROWS, :].rearrange(
                            "(g p) f -> p g f", p=128))
                ml = m % LOAD_MACROS  # macro index within load batch

                h0t = h0p.tile([128, MACRO_ROWS], _DT.float32, tag="h0t")
                h1t = h1p.tile([128, MACRO_ROWS], _DT.float32, tag="h1t")
                h0b = h0s.tile([128, MACRO_ROWS], _DT.bfloat16, tag="h0b")
                h1b = h1s.tile([128, MACRO_ROWS], _DT.bfloat16, tag="h1b")
                h2t = h2p.tile([128, MACRO_FREE], _DT.float32, tag="h2t")

                xt_ps = xtp.tile([128, MACRO_ROWS], _DT.bfloat16,
                                 tag="xt_ps")
                for c in range(CHUNKS_PER_MACRO):
                    g = ml * CHUNKS_PER_MACRO + c
                    nc.tensor.transpose(
                        xt_ps[:, c * 128:(c + 1) * 128],
                        xb[:, g * 128:(g + 1) * 128],
                        ident[:])
                xt = xts.tile([128, MACRO_ROWS], _DT.bfloat16, tag="xt")
                nc.vector.tensor_copy(xt[:], xt_ps[:])
                for j in range(MACRO_IT):
                    nc.tensor.matmul(
                        h0t[:, j * 512:(j + 1) * 512], lhsT=w0[:],
                        rhs=xt[:, j * 512:(j + 1) * 512],
                        start=True, stop=True)

                act_order(nc.scalar.activation(
                    h0b[:], h0t[:], AF.Silu, bias=b0[:, 0:1]))

                for j in range(MACRO_IT):
                    nc.tensor.matmul(
                        h1t[:, j * 512:(j + 1) * 512], lhsT=w1[:],
                        rhs=h0b[:, j * 512:(j + 1) * 512],
                        start=True, stop=True)

                act_order(nc.scalar.activation(
                    h1b[:], h1t[:], AF.Silu, bias=b1[:, 0:1]))

                # mm3 with row-PAIRING: block b = (j, c2, s) covers rows
                # {512j + 256*c2 + 2q + s : q in 0..127}; adjacent s-blocks
                # make each partition's two rows CONSECUTIVE in DRAM, so the
                # store uses 512-byte descriptors instead of 256-byte ones.
                n_mm3 = MACRO_IT * 4 * (2 if with_b2 else 1)
                k = 0
                for j in range(MACRO_IT):
                    for c2 in range(2):
                        for s in range(2):
                            b = j * 4 + c2 * 2 + s
                            lview = (h1b[:, j * 512 + c2 * 256:
                                          j * 512 + (c2 + 1) * 256]
                                     .rearrange("p (q two) -> p q two", two=2)
                                     [:, :, s])
                            nc.tensor.matmul(
                                h2t[:, b * 64:(b + 1) * 64],
                                lhsT=lview, rhs=w2[:],
                                start=(k == 0), stop=(k == n_mm3 - 1))
                            k += 1
                if with_b2:
                    for b in range(MACRO_IT * 4):
                        nc.tensor.matmul(
                            h2t[:, b * 64:(b + 1) * 64],
                            lhsT=ones1[:], rhs=b2[:],
                            start=False, stop=(k == n_mm3 - 1))
                        k += 1

                nc.vector.tensor_copy(pk[:, sb_idx * MACRO_FREE:(sb_idx + 1) * MACRO_FREE],
                                      h2t[:])

                if sb_idx == n_sb - 1:
                    phase_b(pk, sb_base_row, n_sb * MACRO_FREE)
                sb_idx += 1

    return nc


_BUILD_CACHE = {}


def _get_module(nc_rows: int, with_b2: bool, sb_macros: int):
    key = (nc_rows, with_b2, sb_macros)
    if key not in _BUILD_CACHE:
        _BUILD_CACHE[key] = _build(nc_rows, with_b2, sb_macros)
    return _BUILD_CACHE[key]


def kernel(x, edge_index=None, W0=None, b0=None, W1=None, b1=None, W2=None,
           b2=None, **_unused):
    x = np.ascontiguousarray(np.asarray(x), dtype=np.float32)
    n = x.shape[0]
    per = int(math.ceil(n / N_CORES / BATCH_ROWS)) * BATCH_ROWS
    total = per * N_CORES

    xp = np.zeros((total, F_IN), dtype=np.float32)
    xp[:n] = x

    bf = ml_dtypes.bfloat16
    w0b = np.ascontiguousarray(np.asarray(W0, dtype=np.float32)).astype(bf)
    w1b = np.ascontiguousarray(np.asarray(W1, dtype=np.float32)).astype(bf)
    w2b = np.ascontiguousarray(np.asarray(W2, dtype=np.float32)).astype(bf)
    b0f = np.asarray(b0, dtype=np.float32).reshape(F_HID, 1)
    b1f = np.asarray(b1, dtype=np.float32).reshape(F_HID, 1)
    b2f = np.asarray(b2, dtype=np.float32).reshape(1, F_OUT)
    with_b2 = bool(np.any(b2f))
    b2b = b2f.astype(bf)
    ident = np.eye(128, dtype=bf)

    n_macros = per // (IT_ROWS * MACRO_IT)
    sb_macros = min(28, n_macros)

    nc = _get_module(per, with_b2, sb_macros)

    in_maps = []
    for i in range(N_CORES):
        in_maps.append({
            "x": xp[i * per:(i + 1) * per],
            "w0": w0b, "w1": w1b, "w2": w2b,
            "b0": b0f, "b1": b1f, "b2": b2b,
            "ident": ident,
        })

    res = run_bass_kernel_spmd(nc, in_maps, list(range(N_CORES)))
    out = np.concatenate([res.results[i]["out"] for i in range(N_CORES)],
                         axis=0)
    return np.ascontiguousarray(out[:n])


# revision 32
# speedup vs baseline: 417.5048x; 1.0037x over previous
"""Trainium2 Bass kernel for nn_ChebConvNet (ChebConv K=1 => 3-layer MLP + log_softmax).

Computation per node row (edge_index is inert for K=1 ChebConv):
    h = silu(x @ W0 + b0); h = silu(h @ W1 + b1); h2 = h @ W2 + b2
    out = log_softmax(h2, axis=1)

Strategy: shard the 500k node rows across 8 NeuronCores (row-parallel, no
communication). On each core:
  - SWDGE cast-DMA loads x tiles fp32->bf16 row-major.
  - PE transposes 128x128 bf16 chunks to get feature-major x.T (matmul
    contraction must be over features, which requires feature-on-partitions).
  - mm1/mm2 stream activations (weights stationary, bf16, fp32 PSUM accum),
    SiLU on ACT reads PSUM and writes bf16 SBUF (bias folded into ACT bias).
  - mm3 uses h1 chunks as the stationary operand so h2 comes out row-major in
    PSUM; log_softmax then reduces along the free dim.
  - h2 parks in SBUF so exp/ln ACT ops batch into few table-set phases
    (SiLU and Exp/Ln live in different ACT table sets; switches cost ~2.7us).
  - exp without max-subtraction (h2 is O(+-8) here; fp32 exp is exact enough),
    DVE reduce + broadcast subtract, one big row-major store per 2048 rows.
"""

import math
import numpy as np
import ml_dtypes

import bass_rust
import concourse.bass as bass
import concourse.tile as tile
from concourse import mybir
from concourse.bass_utils import run_bass_kernel_spmd
from concourse.vector_clock import ScopedClock
from bass_rust import add_dep_helper

N_CORES = 8
F_IN = 128
F_HID = 128
F_OUT = 64
IT_ROWS = 512          # rows per pipeline iteration
MACRO_IT = 2           # iterations per PSUM macro (1024 rows)
LOAD_MACROS = 2        # macros per DMA load batch (2048 rows)
BATCH_ROWS = IT_ROWS * MACRO_IT * LOAD_MACROS  # 2048
MACRO_ROWS = IT_ROWS * MACRO_IT
CHUNKS_PER_MACRO = MACRO_ROWS // 128       # 128-row transpose chunks
MACRO_FREE = CHUNKS_PER_MACRO * F_OUT      # h2/park free elems per macro
MACRO_BLKS = CHUNKS_PER_MACRO              # 64-wide row blocks per macro
PHASEB_CHUNK = 2048    # free elems per exp/sub op (= 4096 rows)

_DT = mybir.dt

# this walrus build rejects instructions with more than ONE sync wait; the
# Tile framework freely assigns several. Two patches below: (1) split every
# multi-wait instruction by inserting single-wait NoOp carriers on the same
# engine right before it (order on the engine's sequencer preserves
# semantics); (2) the TileContext tail drain gets the same treatment with
# single-wait drain carriers.
_MAX_DRAIN_WAITS = 1
_N_SPARE_DRAINS = 31

_NOOP_CLS = None
_carrier_counter = [0]


def _noop_cls():
    global _NOOP_CLS
    if _NOOP_CLS is None:
        _NOOP_CLS = getattr(bass_rust, "InstNoOp")
    return _NOOP_CLS


_orig_lower_ordered = tile.TileContext._lower_ordered_insts


def _split_multi_waits(self, ordered):
    cls = _noop_cls()
    new_ordered = {}
    for bb_name, insts in ordered.items():
        new_list = []
        for inst in insts:
            si = inst.sync_info
            waits = list(si.on_wait) if si is not None else []
            if len(waits) > 1:
                for w in waits[:-1]:
                    c = cls(name=f"waitcar-{_carrier_counter[0]}", ins=[],
                            outs=[])
                    _carrier_counter[0] += 1
                    c.engine = inst.engine
                    c.sync_info = bass_rust.SyncInfo(on_wait=[w], on_update=[])
                    new_list.append(c)
                inst.sync_info = bass_rust.SyncInfo(
                    on_wait=[waits[-1]], on_update=list(si.on_update))
            new_list.append(inst)
        new_ordered[bb_name] = new_list
    return _orig_lower_ordered(self, new_ordered)


tile.TileContext._lower_ordered_insts = _split_multi_waits


def _patched_drain_and_barrier(self, tick_clock, wait_clock):
    nc = self.nc
    spare = [nc.sync.drain() for _ in range(_N_SPARE_DRAINS)]
    drain_inst = nc.sync.drain()
    wait_clock.add_sem_waits(
        drain_inst.ins, ScopedClock({None: tick_clock.global_clock})
    )
    si = drain_inst.ins.sync_info
    waits = list(si.on_wait) if si is not None else []
    if len(waits) > _MAX_DRAIN_WAITS:
        chunks = [
            waits[i : i + _MAX_DRAIN_WAITS]
            for i in range(0, len(waits), _MAX_DRAIN_WAITS)
        ]
        head, tail = chunks[:-1], chunks[-1]
        assert len(head) <= _N_SPARE_DRAINS, "bump _N_SPARE_DRAINS"
        for nop_i, chunk in zip(spare, head):
            nop_i.ins.sync_info = bass_rust.SyncInfo(on_wait=chunk, on_update=[])
        drain_inst.ins.sync_info = bass_rust.SyncInfo(
            on_wait=tail, on_update=list(si.on_update)
        )
    nc.all_engine_barrier()
    assert self.sems is not None
    popped = nc._tile_sem_poison_stack.pop()
    assert popped is self._sem_poison
    nc.clear_and_free_semaphores(list(self.sems.allocated().values()))
    nc.all_engine_barrier()


tile.TileContext._drain_and_barrier = _patched_drain_and_barrier


def _build(nc_rows: int, with_b2: bool, sb_macros: int):
    """Build the per-core Bass module. nc_rows must be a multiple of 2048."""
    assert nc_rows % BATCH_ROWS == 0
    nc = bass.Bass("TRN2", target_bir_lowering=False, debug=False,
                   num_devices=N_CORES)

    x_d = nc.dram_tensor("x", [nc_rows, F_IN], _DT.float32,
                         kind="ExternalInput").ap()
    w0_d = nc.dram_tensor("w0", [F_IN, F_HID], _DT.bfloat16,
                          kind="ExternalInput").ap()
    w1_d = nc.dram_tensor("w1", [F_HID, F_HID], _DT.bfloat16,
                          kind="ExternalInput").ap()
    w2_d = nc.dram_tensor("w2", [F_HID, F_OUT], _DT.bfloat16,
                          kind="ExternalInput").ap()
    b0_d = nc.dram_tensor("b0", [F_HID, 1], _DT.float32,
                          kind="ExternalInput").ap()
    b1_d = nc.dram_tensor("b1", [F_HID, 1], _DT.float32,
                          kind="ExternalInput").ap()
    b2_d = nc.dram_tensor("b2", [1, F_OUT], _DT.bfloat16,
                          kind="ExternalInput").ap()
    id_d = nc.dram_tensor("ident", [128, 128], _DT.bfloat16,
                          kind="ExternalInput").ap()
    out_d = nc.dram_tensor("out", [nc_rows, F_OUT], _DT.float32,
                           kind="ExternalOutput").ap()

    n_macros = nc_rows // (IT_ROWS * MACRO_IT)
    AF = mybir.ActivationFunctionType

    with tile.TileContext(nc) as tc:
        with (
            tc.tile_pool(name="consts", bufs=1) as consts,
            tc.tile_pool(name="xb", bufs=4) as xpool,
            tc.tile_pool(name="xt_ps", bufs=2, space="PSUM") as xtp,
            tc.tile_pool(name="xt_sb", bufs=2) as xts,
            tc.tile_pool(name="h0_ps", bufs=1, space="PSUM") as h0p,
            tc.tile_pool(name="h1_ps", bufs=1, space="PSUM") as h1p,
            tc.tile_pool(name="h0_sb", bufs=3) as h0s,
            tc.tile_pool(name="h1_sb", bufs=3) as h1s,
            tc.tile_pool(name="h2_ps", bufs=2, space="PSUM") as h2p,
            tc.tile_pool(name="park", bufs=2) as parkp,
            tc.tile_pool(name="e", bufs=2) as epool,
            tc.tile_pool(name="s", bufs=2) as spool,
            tc.tile_pool(name="o", bufs=2) as opool,
        ):
            # ident first: the first PE transposes need it, and HWDGE
            # descriptor generation is serial (~0.6us per dma_start)
            ident = consts.tile([128, 128], _DT.bfloat16, tag="ident")
            nc.sync.dma_start(ident[:], id_d[:, :])
            w0 = consts.tile([128, F_HID], _DT.bfloat16, tag="w0")
            nc.sync.dma_start(w0[:], w0_d[:, :])
            b0 = consts.tile([128, 1], _DT.float32, tag="b0")
            nc.sync.dma_start(b0[:], b0_d[:, :])
            w1 = consts.tile([128, F_HID], _DT.bfloat16, tag="w1")
            nc.sync.dma_start(w1[:], w1_d[:, :])
            b1 = consts.tile([128, 1], _DT.float32, tag="b1")
            nc.sync.dma_start(b1[:], b1_d[:, :])
            w2 = consts.tile([128, F_OUT], _DT.bfloat16, tag="w2")
            nc.sync.dma_start(w2[:], w2_d[:, :])
            b2 = None
            ones1 = None
            if with_b2:
                b2 = consts.tile([1, F_OUT], _DT.bfloat16, tag="b2")
                nc.sync.dma_start(b2[:], b2_d[:, :])
                ones1 = consts.tile([1, 128], _DT.bfloat16, tag="ones1")
                nc.gpsimd.memset(ones1[:], 1.0)

            # chain all ACT instructions in emission order so the scheduler
            # cannot interleave exp/ln (natural_log set) between silu ops
            last_act = [None]

            def act_order(bi):
                if last_act[0] is not None:
                    add_dep_helper(bi.ins, last_act[0].ins, sync=False,
                                   reason="act-table-set phase order")
                last_act[0] = bi

            def phase_b(pk, base_row, width, final=False):
                """softmax tail for parked h2: width free elems (64/row-blk).
                All exps run back-to-back on ACT (no DVE waits in the chain),
                then one batched ln, then GPSIMD subtracts + paired stores.
                The final superbatch has nothing to overlap with, so it uses
                half-size chunks (deeper pipelining) and DVE for the subtract
                (idle in the tail, faster than GPSIMD's 2-input path)."""
                nblk_tot = width // F_OUT
                csize = PHASEB_CHUNK // 2 if final else PHASEB_CHUNK
                chunks = []
                off = 0
                while off < width:
                    w = min(csize, width - off)
                    chunks.append((off, w))
                    off += w
                S = spool.tile([128, sb_macros * MACRO_BLKS], _DT.float32, tag="s")
                LZ = spool.tile([128, sb_macros * MACRO_BLKS], _DT.float32, tag="lz")
                for off, w in chunks:
                    nblk = w // F_OUT
                    e = epool.tile([128, PHASEB_CHUNK], _DT.float32, tag="e")
                    act_order(nc.scalar.activation(
                        e[:, :w], pk[:, off:off + w], AF.Exp))
                    nc.vector.tensor_reduce(
                        S[:, off // F_OUT:off // F_OUT + nblk],
                        e[:, :w].rearrange("p (b f) -> p b f", f=F_OUT),
                        axis=mybir.AxisListType.X, op=mybir.AluOpType.add)
                act_order(nc.scalar.activation(
                    LZ[:, :nblk_tot], S[:, :nblk_tot], AF.Ln))
                for off, w in chunks:
                    nblk = w // F_OUT
                    o = opool.tile([128, PHASEB_CHUNK], _DT.float32, tag="o")
                    lzb = (LZ[:, off // F_OUT:off // F_OUT + nblk]
                           .broadcast_to([128, nblk, F_OUT]))
                    sub_engine = nc.vector if final or not hasattr(
                        nc.gpsimd, "tensor_tensor") else nc.gpsimd
                    sub_engine.tensor_tensor(
                        out=o[:, :w].rearrange("p (b f) -> p b f", f=F_OUT),
                        in0=pk[:, off:off + w].rearrange(
                            "p (b f) -> p b f", f=F_OUT),
                        in1=lzb, op=mybir.AluOpType.subtract)
                    # paired-row layout: block B = 2*P + s holds rows
                    # row0 + 256*P + 2*q + s; (s, f) is 512B-contiguous in DRAM
                    row0 = base_row + (off // F_OUT) * 128
                    nrows = nblk * 128
                    nc.sync.dma_start(
                        out_d[row0:row0 + nrows, :].rearrange(
                            "(P q s) f -> q P s f", q=128, s=2),
                        o[:, :w].rearrange("p (P s f) -> p P s f",
                                           s=2, f=F_OUT))

            # superbatch schedule: full-size SBs, then a short final SB so
            # the last (un-overlapped) phase-B tail is small
            TAIL_SB = max(2, sb_macros // 3)
            sb_sizes = []
            rem = n_macros
            while rem > 0:
                if rem <= sb_macros:
                    sb_sizes.append(rem); rem = 0
                elif rem <= sb_macros + TAIL_SB:
                    sb_sizes.append(rem - TAIL_SB); sb_sizes.append(TAIL_SB)
                    rem = 0
                else:
                    sb_sizes.append(sb_macros); rem -= sb_macros
            sb_bounds = []
            acc = 0
            for sz in sb_sizes:
                sb_bounds.append((acc, sz)); acc += sz
            sb_start = {st: sz for st, sz in sb_bounds}

            xb = None
            sb_idx = 0
            for m in range(n_macros):
                if m in sb_start:
                    n_sb = sb_start[m]
                    sb_idx = 0
                    pk = parkp.tile([128, sb_macros * MACRO_FREE], _DT.float32,
                                    tag="park")
                    sb_base_row = m * IT_ROWS * MACRO_IT

                if m % LOAD_MACROS == 0:
                    xb = xpool.tile([128, BATCH_ROWS], _DT.bfloat16, tag="xb")
                    r0 = m * IT_ROWS * MACRO_IT
                    nc.gpsimd.dma_start(
                        xb[:].rearrange("p (g f) -> p g f", f=F_IN),
                        x_d[r0:r0 + BATCH_ROWS, :].rearrange(
                            "(g p) f -> p g f", p=128))
                ml = m % LOAD_MACROS  # macro index within load batch

                h0t = h0p.tile([128, MACRO_ROWS], _DT.float32, tag="h0t")
                h1t = h1p.tile([128, MACRO_ROWS], _DT.float32, tag="h1t")
                h0b = h0s.tile([128, MACRO_ROWS], _DT.bfloat16, tag="h0b")
                h1b = h1s.tile([128, MACRO_ROWS], _DT.bfloat16, tag="h1b")
                h2t = h2p.tile([128, MACRO_FREE], _DT.float32, tag="h2t")

                xt_ps = xtp.tile([128, MACRO_ROWS], _DT.bfloat16,
                                 tag="xt_ps")
                for c in range(CHUNKS_PER_MACRO):
                    g = ml * CHUNKS_PER_MACRO + c
                    nc.tensor.transpose(
                        xt_ps[:, c * 128:(c + 1) * 128],
                        xb[:, g * 128:(g + 1) * 128],
                        ident[:])
                xt = xts.tile([128, MACRO_ROWS], _DT.bfloat16, tag="xt")
                nc.vector.tensor_copy(xt[:], xt_ps[:])
                for j in range(MACRO_IT):
                    nc.tensor.matmul(
                        h0t[:, j * 512:(j + 1) * 512], lhsT=w0[:],
                        rhs=xt[:, j * 512:(j + 1) * 512],
                        start=True, stop=True)

                act_order(nc.scalar.activation(
                    h0b[:], h0t[:], AF.Silu, bias=b0[:, 0:1]))

                for j in range(MACRO_IT):
                    nc.tensor.matmul(
                        h1t[:, j * 512:(j + 1) * 512], lhsT=w1[:],
                        rhs=h0b[:, j * 512:(j + 1) * 512],
                        start=True, stop=True)

                act_order(nc.scalar.activation(
                    h1b[:], h1t[:], AF.Silu, bias=b1[:, 0:1]))

                # mm3 with row-PAIRING: block b = (j, c2, s) covers rows
                # {512j + 256*c2 + 2q + s : q in 0..127}; adjacent s-blocks
                # make each partition's two rows CONSECUTIVE in DRAM, so the
                # store uses 512-byte descriptors instead of 256-byte ones.
                n_mm3 = MACRO_IT * 4 * (2 if with_b2 else 1)
                k = 0
                for j in range(MACRO_IT):
                    for c2 in range(2):
                        for s in range(2):
                            b = j * 4 + c2 * 2 + s
                            lview = (h1b[:, j * 512 + c2 * 256:
                                          j * 512 + (c2 + 1) * 256]
                                     .rearrange("p (q two) -> p q two", two=2)
                                     [:, :, s])
                            nc.tensor.matmul(
                                h2t[:, b * 64:(b + 1) * 64],
                                lhsT=lview, rhs=w2[:],
                                start=(k == 0), stop=(k == n_mm3 - 1))
                            k += 1
                if with_b2:
                    for b in range(MACRO_IT * 4):
                        nc.tensor.matmul(
                            h2t[:, b * 64:(b + 1) * 64],
                            lhsT=ones1[:], rhs=b2[:],
                            start=False, stop=(k == n_mm3 - 1))
                        k += 1

                nc.vector.tensor_copy(pk[:, sb_idx * MACRO_FREE:(sb_idx + 1) * MACRO_FREE],
                                      h2t[:])

                if sb_idx == n_sb - 1:
                    phase_b(pk, sb_base_row, n_sb * MACRO_FREE,
                            final=(m == n_macros - 1))
                sb_idx += 1

    return nc


_BUILD_CACHE = {}


def _get_module(nc_rows: int, with_b2: bool, sb_macros: int):
    key = (nc_rows, with_b2, sb_macros)
    if key not in _BUILD_CACHE:
        _BUILD_CACHE[key] = _build(nc_rows, with_b2, sb_macros)
    return _BUILD_CACHE[key]


def kernel(x, edge_index=None, W0=None, b0=None, W1=None, b1=None, W2=None,
           b2=None, **_unused):
    x = np.ascontiguousarray(np.asarray(x), dtype=np.float32)
    n = x.shape[0]
    per = int(math.ceil(n / N_CORES / BATCH_ROWS)) * BATCH_ROWS
    total = per * N_CORES

    xp = np.zeros((total, F_IN), dtype=np.float32)
    xp[:n] = x

    bf = ml_dtypes.bfloat16
    w0b = np.ascontiguousarray(np.asarray(W0, dtype=np.float32)).astype(bf)
    w1b = np.ascontiguousarray(np.asarray(W1, dtype=np.float32)).astype(bf)
    w2b = np.ascontiguousarray(np.asarray(W2, dtype=np.float32)).astype(bf)
    b0f = np.asarray(b0, dtype=np.float32).reshape(F_HID, 1)
    b1f = np.asarray(b1, dtype=np.float32).reshape(F_HID, 1)
    b2f = np.asarray(b2, dtype=np.float32).reshape(1, F_OUT)
    with_b2 = bool(np.any(b2f))
    b2b = b2f.astype(bf)
    ident = np.eye(128, dtype=bf)

    n_macros = per // (IT_ROWS * MACRO_IT)
    sb_macros = min(28, n_macros)

    nc = _get_module(per, with_b2, sb_macros)

    in_maps = []
    for i in range(N_CORES):
        in_maps.append({
            "x": xp[i * per:(i + 1) * per],
            "w0": w0b, "w1": w1b, "w2": w2b,
            "b0": b0f, "b1": b1f, "b2": b2b,
            "ident": ident,
        })

    res = run_bass_kernel_spmd(nc, in_maps, list(range(N_CORES)))
    out = np.concatenate([res.results[i]["out"] for i in range(N_CORES)],
                         axis=0)
    return np.ascontiguousarray(out[:n])


# revision 34
# speedup vs baseline: 417.9863x; 1.0012x over previous
"""Trainium2 Bass kernel for nn_ChebConvNet (ChebConv K=1 => 3-layer MLP + log_softmax).

Computation per node row (edge_index is inert for K=1 ChebConv):
    h = silu(x @ W0 + b0); h = silu(h @ W1 + b1); h2 = h @ W2 + b2
    out = log_softmax(h2, axis=1)

Strategy: shard the 500k node rows across 8 NeuronCores (row-parallel, no
communication). On each core:
  - SWDGE cast-DMA loads x tiles fp32->bf16 row-major.
  - PE transposes 128x128 bf16 chunks to get feature-major x.T (matmul
    contraction must be over features, which requires feature-on-partitions).
  - mm1/mm2 stream activations (weights stationary, bf16, fp32 PSUM accum),
    SiLU on ACT reads PSUM and writes bf16 SBUF (bias folded into ACT bias).
  - mm3 uses h1 chunks as the stationary operand so h2 comes out row-major in
    PSUM; log_softmax then reduces along the free dim.
  - h2 parks in SBUF so exp/ln ACT ops batch into few table-set phases
    (SiLU and Exp/Ln live in different ACT table sets; switches cost ~2.7us).
  - exp without max-subtraction (h2 is O(+-8) here; fp32 exp is exact enough),
    DVE reduce + broadcast subtract, one big row-major store per 2048 rows.
"""

import math
import numpy as np
import ml_dtypes

import bass_rust
import concourse.bass as bass
import concourse.tile as tile
from concourse import mybir
from concourse.bass_utils import run_bass_kernel_spmd
from concourse.vector_clock import ScopedClock
from bass_rust import add_dep_helper

N_CORES = 8
F_IN = 128
F_HID = 128
F_OUT = 64
IT_ROWS = 512          # rows per pipeline iteration
MACRO_IT = 2           # iterations per PSUM macro (1024 rows)
LOAD_MACROS = 2        # macros per DMA load batch (2048 rows)
BATCH_ROWS = IT_ROWS * MACRO_IT * LOAD_MACROS  # 2048
MACRO_ROWS = IT_ROWS * MACRO_IT
CHUNKS_PER_MACRO = MACRO_ROWS // 128       # 128-row transpose chunks
MACRO_FREE = CHUNKS_PER_MACRO * F_OUT      # h2/park free elems per macro
MACRO_BLKS = CHUNKS_PER_MACRO              # 64-wide row blocks per macro
PHASEB_CHUNK = 2048    # free elems per exp/sub op (= 4096 rows)

_DT = mybir.dt

# this walrus build rejects instructions with more than ONE sync wait; the
# Tile framework freely assigns several. Two patches below: (1) split every
# multi-wait instruction by inserting single-wait NoOp carriers on the same
# engine right before it (order on the engine's sequencer preserves
# semantics); (2) the TileContext tail drain gets the same treatment with
# single-wait drain carriers.
_MAX_DRAIN_WAITS = 1
_N_SPARE_DRAINS = 31

_NOOP_CLS = None
_carrier_counter = [0]


def _noop_cls():
    global _NOOP_CLS
    if _NOOP_CLS is None:
        _NOOP_CLS = getattr(bass_rust, "InstNoOp")
    return _NOOP_CLS


_orig_lower_ordered = tile.TileContext._lower_ordered_insts


def _split_multi_waits(self, ordered):
    cls = _noop_cls()
    new_ordered = {}
    for bb_name, insts in ordered.items():
        new_list = []
        for inst in insts:
            si = inst.sync_info
            waits = list(si.on_wait) if si is not None else []
            if len(waits) > 1:
                for w in waits[:-1]:
                    c = cls(name=f"waitcar-{_carrier_counter[0]}", ins=[],
                            outs=[])
                    _carrier_counter[0] += 1
                    c.engine = inst.engine
                    c.sync_info = bass_rust.SyncInfo(on_wait=[w], on_update=[])
                    new_list.append(c)
                inst.sync_info = bass_rust.SyncInfo(
                    on_wait=[waits[-1]], on_update=list(si.on_update))
            new_list.append(inst)
        new_ordered[bb_name] = new_list
    return _orig_lower_ordered(self, new_ordered)


tile.TileContext._lower_ordered_insts = _split_multi_waits


def _patched_drain_and_barrier(self, tick_clock, wait_clock):
    nc = self.nc
    spare = [nc.sync.drain() for _ in range(_N_SPARE_DRAINS)]
    drain_inst = nc.sync.drain()
    wait_clock.add_sem_waits(
        drain_inst.ins, ScopedClock({None: tick_clock.global_clock})
    )
    si = drain_inst.ins.sync_info
    waits = list(si.on_wait) if si is not None else []
    if len(waits) > _MAX_DRAIN_WAITS:
        chunks = [
            waits[i : i + _MAX_DRAIN_WAITS]
            for i in range(0, len(waits), _MAX_DRAIN_WAITS)
        ]
        head, tail = chunks[:-1], chunks[-1]
        assert len(head) <= _N_SPARE_DRAINS, "bump _N_SPARE_DRAINS"
        for nop_i, chunk in zip(spare, head):
            nop_i.ins.sync_info = bass_rust.SyncInfo(on_wait=chunk, on_update=[])
        drain_inst.ins.sync_info = bass_rust.SyncInfo(
            on_wait=tail, on_update=list(si.on_update)
        )
    nc.all_engine_barrier()
    assert self.sems is not None
    popped = nc._tile_sem_poison_stack.pop()
    assert popped is self._sem_poison
    nc.clear_and_free_semaphores(list(self.sems.allocated().values()))
    nc.all_engine_barrier()


tile.TileContext._drain_and_barrier = _patched_drain_and_barrier


def _build(nc_rows: int, with_b2: bool, sb_macros: int):
    """Build the per-core Bass module. nc_rows must be a multiple of 2048."""
    assert nc_rows % BATCH_ROWS == 0
    nc = bass.Bass("TRN2", target_bir_lowering=False, debug=False,
                   num_devices=N_CORES)

    x_d = nc.dram_tensor("x", [nc_rows, F_IN], _DT.float32,
                         kind="ExternalInput").ap()
    w0_d = nc.dram_tensor("w0", [F_IN, F_HID], _DT.bfloat16,
                          kind="ExternalInput").ap()
    w1_d = nc.dram_tensor("w1", [F_HID, F_HID], _DT.bfloat16,
                          kind="ExternalInput").ap()
    w2_d = nc.dram_tensor("w2", [F_HID, F_OUT], _DT.bfloat16,
                          kind="ExternalInput").ap()
    b0_d = nc.dram_tensor("b0", [F_HID, 1], _DT.float32,
                          kind="ExternalInput").ap()
    b1_d = nc.dram_tensor("b1", [F_HID, 1], _DT.float32,
                          kind="ExternalInput").ap()
    b2_d = nc.dram_tensor("b2", [1, F_OUT], _DT.bfloat16,
                          kind="ExternalInput").ap()
    id_d = nc.dram_tensor("ident", [128, 128], _DT.bfloat16,
                          kind="ExternalInput").ap()
    out_d = nc.dram_tensor("out", [nc_rows, F_OUT], _DT.float32,
                           kind="ExternalOutput").ap()

    n_macros = nc_rows // (IT_ROWS * MACRO_IT)
    AF = mybir.ActivationFunctionType

    with tile.TileContext(nc) as tc:
        with (
            tc.tile_pool(name="consts", bufs=1) as consts,
            tc.tile_pool(name="xb", bufs=4) as xpool,
            tc.tile_pool(name="xt_ps", bufs=2, space="PSUM") as xtp,
            tc.tile_pool(name="xt_sb", bufs=2) as xts,
            tc.tile_pool(name="h0_ps", bufs=1, space="PSUM") as h0p,
            tc.tile_pool(name="h1_ps", bufs=1, space="PSUM") as h1p,
            tc.tile_pool(name="h0_sb", bufs=3) as h0s,
            tc.tile_pool(name="h1_sb", bufs=3) as h1s,
            tc.tile_pool(name="h2_ps", bufs=2, space="PSUM") as h2p,
            tc.tile_pool(name="park", bufs=2) as parkp,
            tc.tile_pool(name="e", bufs=2) as epool,
            tc.tile_pool(name="s", bufs=2) as spool,
            tc.tile_pool(name="o", bufs=2) as opool,
        ):
            # ident first: the first PE transposes need it, and HWDGE
            # descriptor generation is serial (~0.6us per dma_start)
            ident = consts.tile([128, 128], _DT.bfloat16, tag="ident")
            nc.sync.dma_start(ident[:], id_d[:, :])
            w0 = consts.tile([128, F_HID], _DT.bfloat16, tag="w0")
            nc.sync.dma_start(w0[:], w0_d[:, :])
            b0 = consts.tile([128, 1], _DT.float32, tag="b0")
            nc.sync.dma_start(b0[:], b0_d[:, :])
            w1 = consts.tile([128, F_HID], _DT.bfloat16, tag="w1")
            nc.sync.dma_start(w1[:], w1_d[:, :])
            b1 = consts.tile([128, 1], _DT.float32, tag="b1")
            nc.sync.dma_start(b1[:], b1_d[:, :])
            w2 = consts.tile([128, F_OUT], _DT.bfloat16, tag="w2")
            nc.sync.dma_start(w2[:], w2_d[:, :])
            b2 = None
            ones1 = None
            if with_b2:
                b2 = consts.tile([1, F_OUT], _DT.bfloat16, tag="b2")
                nc.sync.dma_start(b2[:], b2_d[:, :])
                ones1 = consts.tile([1, 128], _DT.bfloat16, tag="ones1")
                nc.gpsimd.memset(ones1[:], 1.0)

            # chain all ACT instructions in emission order so the scheduler
            # cannot interleave exp/ln (natural_log set) between silu ops
            last_act = [None]

            def act_order(bi):
                if last_act[0] is not None:
                    add_dep_helper(bi.ins, last_act[0].ins, sync=False,
                                   reason="act-table-set phase order")
                last_act[0] = bi

            def phase_b(pk, base_row, width, final=False):
                """softmax tail for parked h2: width free elems (64/row-blk).
                All exps run back-to-back on ACT (no DVE waits in the chain),
                then one batched ln, then GPSIMD subtracts + paired stores.
                The final superbatch has nothing to overlap with, so it uses
                half-size chunks (deeper pipelining) and DVE for the subtract
                (idle in the tail, faster than GPSIMD's 2-input path)."""
                nblk_tot = width // F_OUT
                csize = PHASEB_CHUNK // 2 if final else PHASEB_CHUNK
                chunks = []
                off = 0
                while off < width:
                    w = min(csize, width - off)
                    chunks.append((off, w))
                    off += w
                S = spool.tile([128, sb_macros * MACRO_BLKS], _DT.float32, tag="s")
                LZ = spool.tile([128, sb_macros * MACRO_BLKS], _DT.float32, tag="lz")
                for off, w in chunks:
                    nblk = w // F_OUT
                    e = epool.tile([128, PHASEB_CHUNK], _DT.float32, tag="e")
                    act_order(nc.scalar.activation(
                        e[:, :w], pk[:, off:off + w], AF.Exp))
                    nc.vector.tensor_reduce(
                        S[:, off // F_OUT:off // F_OUT + nblk],
                        e[:, :w].rearrange("p (b f) -> p b f", f=F_OUT),
                        axis=mybir.AxisListType.X, op=mybir.AluOpType.add)
                act_order(nc.scalar.activation(
                    LZ[:, :nblk_tot], S[:, :nblk_tot], AF.Ln))
                for off, w in chunks:
                    nblk = w // F_OUT
                    o = opool.tile([128, PHASEB_CHUNK], _DT.float32, tag="o")
                    lzb = (LZ[:, off // F_OUT:off // F_OUT + nblk]
                           .broadcast_to([128, nblk, F_OUT]))
                    sub_engine = nc.vector if final or not hasattr(
                        nc.gpsimd, "tensor_tensor") else nc.gpsimd
                    sub_engine.tensor_tensor(
                        out=o[:, :w].rearrange("p (b f) -> p b f", f=F_OUT),
                        in0=pk[:, off:off + w].rearrange(
                            "p (b f) -> p b f", f=F_OUT),
                        in1=lzb, op=mybir.AluOpType.subtract)
                    # paired-row layout: block B = 2*P + s holds rows
                    # row0 + 256*P + 2*q + s; (s, f) is 512B-contiguous in DRAM
                    row0 = base_row + (off // F_OUT) * 128
                    nrows = nblk * 128
                    nc.sync.dma_start(
                        out_d[row0:row0 + nrows, :].rearrange(
                            "(P q s) f -> q P s f", q=128, s=2),
                        o[:, :w].rearrange("p (P s f) -> p P s f",
                                           s=2, f=F_OUT))

            # superbatch schedule: full-size SBs, then a short final SB so
            # the last (un-overlapped) phase-B tail is small
            TAIL_SB = max(2, sb_macros // 3)
            sb_sizes = []
            rem = n_macros
            while rem > 0:
                if rem <= sb_macros:
                    sb_sizes.append(rem); rem = 0
                elif rem <= sb_macros + TAIL_SB:
                    sb_sizes.append(rem - TAIL_SB); sb_sizes.append(TAIL_SB)
                    rem = 0
                else:
                    sb_sizes.append(sb_macros); rem -= sb_macros
            sb_bounds = []
            acc = 0
            for sz in sb_sizes:
                sb_bounds.append((acc, sz)); acc += sz
            sb_start = {st: sz for st, sz in sb_bounds}

            xb = None
            sb_idx = 0
            for m in range(n_macros):
                if m in sb_start:
                    n_sb = sb_start[m]
                    sb_idx = 0
                    pk = parkp.tile([128, sb_macros * MACRO_FREE], _DT.float32,
                                    tag="park")
                    sb_base_row = m * IT_ROWS * MACRO_IT

                if m % LOAD_MACROS == 0:
                    xb = xpool.tile([128, BATCH_ROWS], _DT.bfloat16, tag="xb")
                    r0 = m * IT_ROWS * MACRO_IT
                    # split the very first load so macro 0's transposes can
                    # start as soon as the first half lands (shorter ramp)
                    parts = 2 if m == 0 else 1
                    half = BATCH_ROWS // parts
                    for pi in range(parts):
                        nc.gpsimd.dma_start(
                            xb[:, pi * half:(pi + 1) * half].rearrange(
                                "p (g f) -> p g f", f=F_IN),
                            x_d[r0 + pi * half:r0 + (pi + 1) * half, :]
                            .rearrange("(g p) f -> p g f", p=128))
                ml = m % LOAD_MACROS  # macro index within load batch

                h0t = h0p.tile([128, MACRO_ROWS], _DT.float32, tag="h0t")
                h1t = h1p.tile([128, MACRO_ROWS], _DT.float32, tag="h1t")
                h0b = h0s.tile([128, MACRO_ROWS], _DT.bfloat16, tag="h0b")
                h1b = h1s.tile([128, MACRO_ROWS], _DT.bfloat16, tag="h1b")
                h2t = h2p.tile([128, MACRO_FREE], _DT.float32, tag="h2t")

                xt_ps = xtp.tile([128, MACRO_ROWS], _DT.bfloat16,
                                 tag="xt_ps")
                for c in range(CHUNKS_PER_MACRO):
                    g = ml * CHUNKS_PER_MACRO + c
                    nc.tensor.transpose(
                        xt_ps[:, c * 128:(c + 1) * 128],
                        xb[:, g * 128:(g + 1) * 128],
                        ident[:])
                xt = xts.tile([128, MACRO_ROWS], _DT.bfloat16, tag="xt")
                nc.vector.tensor_copy(xt[:], xt_ps[:])
                for j in range(MACRO_IT):
                    nc.tensor.matmul(
                        h0t[:, j * 512:(j + 1) * 512], lhsT=w0[:],
                        rhs=xt[:, j * 512:(j + 1) * 512],
                        start=True, stop=True)

                act_order(nc.scalar.activation(
                    h0b[:], h0t[:], AF.Silu, bias=b0[:, 0:1]))

                for j in range(MACRO_IT):
                    nc.tensor.matmul(
                        h1t[:, j * 512:(j + 1) * 512], lhsT=w1[:],
                        rhs=h0b[:, j * 512:(j + 1) * 512],
                        start=True, stop=True)

                act_order(nc.scalar.activation(
                    h1b[:], h1t[:], AF.Silu, bias=b1[:, 0:1]))

                # mm3 with row-PAIRING: block b = (j, c2, s) covers rows
                # {512j + 256*c2 + 2q + s : q in 0..127}; adjacent s-blocks
                # make each partition's two rows CONSECUTIVE in DRAM, so the
                # store uses 512-byte descriptors instead of 256-byte ones.
                n_mm3 = MACRO_IT * 4 * (2 if with_b2 else 1)
                k = 0
                for j in range(MACRO_IT):
                    for c2 in range(2):
                        for s in range(2):
                            b = j * 4 + c2 * 2 + s
                            lview = (h1b[:, j * 512 + c2 * 256:
                                          j * 512 + (c2 + 1) * 256]
                                     .rearrange("p (q two) -> p q two", two=2)
                                     [:, :, s])
                            nc.tensor.matmul(
                                h2t[:, b * 64:(b + 1) * 64],
                                lhsT=lview, rhs=w2[:],
                                start=(k == 0), stop=(k == n_mm3 - 1))
                            k += 1
                if with_b2:
                    for b in range(MACRO_IT * 4):
                        nc.tensor.matmul(
                            h2t[:, b * 64:(b + 1) * 64],
                            lhsT=ones1[:], rhs=b2[:],
                            start=False, stop=(k == n_mm3 - 1))
                        k += 1

                nc.vector.tensor_copy(pk[:, sb_idx * MACRO_FREE:(sb_idx + 1) * MACRO_FREE],
                                      h2t[:])

                if sb_idx == n_sb - 1:
                    phase_b(pk, sb_base_row, n_sb * MACRO_FREE,
                            final=(m == n_macros - 1))
                sb_idx += 1

    return nc


_BUILD_CACHE = {}


def _get_module(nc_rows: int, with_b2: bool, sb_macros: int):
    key = (nc_rows, with_b2, sb_macros)
    if key not in _BUILD_CACHE:
        _BUILD_CACHE[key] = _build(nc_rows, with_b2, sb_macros)
    return _BUILD_CACHE[key]


def kernel(x, edge_index=None, W0=None, b0=None, W1=None, b1=None, W2=None,
           b2=None, **_unused):
    x = np.ascontiguousarray(np.asarray(x), dtype=np.float32)
    n = x.shape[0]
    per = int(math.ceil(n / N_CORES / BATCH_ROWS)) * BATCH_ROWS
    total = per * N_CORES

    xp = np.zeros((total, F_IN), dtype=np.float32)
    xp[:n] = x

    bf = ml_dtypes.bfloat16
    w0b = np.ascontiguousarray(np.asarray(W0, dtype=np.float32)).astype(bf)
    w1b = np.ascontiguousarray(np.asarray(W1, dtype=np.float32)).astype(bf)
    w2b = np.ascontiguousarray(np.asarray(W2, dtype=np.float32)).astype(bf)
    b0f = np.asarray(b0, dtype=np.float32).reshape(F_HID, 1)
    b1f = np.asarray(b1, dtype=np.float32).reshape(F_HID, 1)
    b2f = np.asarray(b2, dtype=np.float32).reshape(1, F_OUT)
    with_b2 = bool(np.any(b2f))
    b2b = b2f.astype(bf)
    ident = np.eye(128, dtype=bf)

    n_macros = per // (IT_ROWS * MACRO_IT)
    sb_macros = min(28, n_macros)

    nc = _get_module(per, with_b2, sb_macros)

    in_maps = []
    for i in range(N_CORES):
        in_maps.append({
            "x": xp[i * per:(i + 1) * per],
            "w0": w0b, "w1": w1b, "w2": w2b,
            "b0": b0f, "b1": b1f, "b2": b2b,
            "ident": ident,
        })

    res = run_bass_kernel_spmd(nc, in_maps, list(range(N_CORES)))
    out = np.concatenate([res.results[i]["out"] for i in range(N_CORES)],
                         axis=0)
    return np.ascontiguousarray(out[:n])
